# revision 7
# baseline (speedup 1.0000x reference)
"""Trainium2 kernel for PhysicalDiffraction:
    out = real(ifft2(fft2(x) * (H_real + i*H_imag)))   x: [8,16,512,512] f32

Method (Hartley / DHT formulation)
----------------------------------
The op is a real->real linear map: out = x (*) h_r with h_r =
real(ifft2(Hc)) a real circular kernel, i.e. the filter can be
Hermitian-symmetrized G = (Hc + conj(Hc[-u,-v]))/2 = fft2(h_r).

Using the separable row-column discrete Hartley transform
A = CASP @ x @ CASP (CASP = C + S, cas matrix), the convolution theorem
is   Z = A.E1 + A(-u,v).E2 + A(u,-v).E3 + A(-u,-v).E4
with E1..E4 host-precomputed from G.  Writing each reversed term as a
reversal of a plain product (Ru(A.Ru E2) etc.) and noting that a
u-reversal of a matmul's lhsT is equivalent to swapping the DFT weight
matrix CASP <-> CASM (CASM = C - S, since cas(-x) = cos - sin), ALL
index reversals get absorbed into which weight matrix the inverse
stages use.  Per image (all real matmuls, 512^3):

  S1: S1p = x^T CASP                          (1 unit)
  S2: A   = S1p^T CASP                        (1 unit)
  S3: m_i = A . E_i  (i=1..4, VectorE, fused PSUM evacuation)
  S4: Ta = m1^T CASP + m2^T CASM              (2 units)
      Tb = m3^T CASP + m4^T CASM
  S5: out = Ta^T CASP + Tb^T CASM             (2 units)

6 matmul units/image vs 8 for complex-packing -> ~25% less TensorE
work, no transposes, no cross-partition moves.  Matmuls in float32r
(full speed, ~1e-4 relative).  The lhsT=data trick makes each stage
transform + transpose in one go.

Sharding: batch*channel = 128 images, 16 per NeuronCore, data-parallel.

The walrus build here accepts only ONE semaphore wait per instruction;
Tile emits more.  `legalize_waits` splits excess waits onto same-engine
NoOps inserted just before the instruction (the engine sequencer stalls
there instead), which preserves semantics.
"""

import sys

for _p in ("/opt/trn_rl_repo", "/root/.axon_site/_ro/trn_rl_repo"):
    if _p not in sys.path:
        sys.path.append(_p)

import numpy as np
from concourse import bass, mybir
from concourse.tile import TileContext
from concourse.bass_utils import run_bass_kernel_spmd

N = 512
NCHUNK = N // 128  # 4
N_CORES = 8
IMG_PER_CORE = 16

MM_DTYPE = mybir.dt.float32r


# ---------------------------------------------------------------------------
# wait legalizer
# ---------------------------------------------------------------------------
_counter = [0]


def _fresh_name():
    _counter[0] += 1
    return f"I-waitfix-{_counter[0]}"


def legalize_waits(nc, limit=1):
    for fn in nc.m.functions:
        for blk in fn.blocks:
            out = []
            changed = False
            for inst in blk.instructions:
                si = inst.sync_info
                waits = list(si.on_wait) if si is not None and si.on_wait else []
                if len(waits) > limit:
                    excess = waits[: len(waits) - limit]
                    keep = waits[len(waits) - limit :]
                    for w in excess:
                        out.append(
                            mybir.InstNoOp(
                                name=_fresh_name(),
                                engine=inst.engine,
                                ins=[],
                                outs=[],
                                sync_info=mybir.SyncInfo(on_wait=[w], on_update=[]),
                            )
                        )
                    inst.sync_info = mybir.SyncInfo(
                        on_wait=keep,
                        on_update=list(si.on_update) if si.on_update else [],
                    )
                    changed = True
                out.append(inst)
            if changed:
                blk.instructions = out
    return nc


# ---------------------------------------------------------------------------
# bass program: one core, n_img images
# ---------------------------------------------------------------------------
def _plane(dram_ap):
    """[512,512] DRAM view -> [128, 4, 512] partition-major AP."""
    return dram_ap.rearrange("(k p) w -> p k w", p=128)


def build_nc(n_img=IMG_PER_CORE, mm_dtype=MM_DTYPE):
    f32 = mybir.dt.float32
    nc = bass.Bass()
    x = nc.declare_dram_parameter("x", [n_img, N, N], f32, isOutput=False)
    wmat = nc.declare_dram_parameter("wmat", [2, N, N], f32, isOutput=False)
    emat = nc.declare_dram_parameter("emat", [4, N, N], f32, isOutput=False)
    out = nc.declare_dram_parameter("out", [n_img, N, N], f32, isOutput=True)

    shp = [128, NCHUNK, N]

    with TileContext(nc) as tc:
        with (
            tc.tile_pool(name="wts", bufs=1) as wts,
            tc.tile_pool(name="stg", bufs=3) as stg,
            tc.tile_pool(name="xin", bufs=2) as xin,
            tc.tile_pool(name="mid", bufs=1) as mid,
            tc.tile_pool(name="outp", bufs=2) as outp,
            tc.tile_pool(name="ps", bufs=8, space="PSUM") as ps,
        ):
            # --- one-time: CASP + REV weights (rounded to mm dtype), E planes ---
            caspt = wts.tile(shp, mm_dtype, tag="caspt")
            revt = wts.tile(shp, mm_dtype, tag="revt")
            for i, dst in ((0, caspt), (1, revt)):
                s = stg.tile(shp, f32, tag="stg", name=f"wstg{i}")
                nc.sync.dma_start(out=s[:], in_=_plane(wmat[i]))
                nc.vector.tensor_copy(dst[:], s[:])
            et = []
            for i in range(4):
                t = wts.tile(shp, f32, tag=f"e{i}", name=f"e{i}")
                nc.sync.dma_start(out=t[:], in_=_plane(emat[i]))
                et.append(t)

            def acc_mm(bank, terms, mt):
                """bank = sum_terms lhsT_plane[:,k,mt]^T @ rhs[:,k,:]"""
                nterm = len(terms)
                for ti, (plane, rhs) in enumerate(terms):
                    for k in range(NCHUNK):
                        nc.tensor.matmul(
                            bank[:, :],
                            plane[:, k, mt * 128 : (mt + 1) * 128],
                            rhs[:, k, :],
                            start=(ti == 0 and k == 0),
                            stop=(ti == nterm - 1 and k == NCHUNK - 1),
                        )

            for j in range(n_img):
                # --- stage input (cast/round to mm dtype on DVE) ---
                xr = xin.tile(shp, mm_dtype, tag="xr")
                s = stg.tile(shp, f32, tag="stg", name=f"xstg{j}")
                nc.sync.dma_start(out=s[:], in_=_plane(x[j]))
                nc.vector.tensor_copy(xr[:], s[:])

                # --- S1: S1p = x^T CASP ---
                s1p = mid.tile(shp, mm_dtype, tag="s1p", bufs=2)
                for m in range(NCHUNK):
                    b = ps.tile([128, N], f32, tag="bank")
                    acc_mm(b, [(xr, caspt)], m)
                    nc.scalar.copy(s1p[:, m, :], b[:, :])

                # --- S2: A = S1p^T CASP (DHT of image) ---
                at = mid.tile(shp, mm_dtype, tag="at", bufs=2)
                for m in range(NCHUNK):
                    b = ps.tile([128, N], f32, tag="bank")
                    acc_mm(b, [(s1p, caspt)], m)
                    nc.scalar.copy(at[:, m, :], b[:, :])

                # m1 = A*E1, m3' = A*RvE3 (GpSimd, overlaps DVE/PE)
                tpl = mid.tile(shp, f32, tag="tpl")
                spl = mid.tile(shp, f32, tag="spl")
                af = at[:].bitcast(mybir.dt.float32)
                nc.vector.tensor_mul(tpl[:], af, et[0][:])
                nc.vector.tensor_mul(spl[:], af, et[2][:])

                # --- REV: RuA = REV^T A ; evac fused with E2/RvE4 mults ---
                m2pl = mid.tile(shp, f32, tag="m2pl")
                m4pl = mid.tile(shp, f32, tag="m4pl")
                for m in range(NCHUNK):
                    b = ps.tile([128, N], f32, tag="bank")
                    acc_mm(b, [(revt, at)], m)
                    nc.vector.tensor_mul(m2pl[:, m, :], b[:, :], et[1][:, m, :])
                    nc.vector.tensor_mul(m4pl[:, m, :], b[:, :], et[3][:, m, :])

                # u = m1+m2 ; v = m3'+m4' ; Z = u + Rv(v)
                upl = stg.tile(shp, f32, tag="stg", name=f"u{j}")
                vpl = stg.tile(shp, f32, tag="stg", name=f"v{j}")
                nc.gpsimd.tensor_add(upl[:], tpl[:], m2pl[:])
                nc.vector.tensor_add(vpl[:], spl[:], m4pl[:])
                zt = mid.tile(shp, mm_dtype, tag="zt", bufs=2)
                nc.vector.tensor_add(
                    zt[:, :, 1:N], upl[:, :, 1:N], vpl[:, :, N - 1 : 0 : -1]
                )
                nc.vector.tensor_add(
                    zt[:, :, 0:1], upl[:, :, 0:1], vpl[:, :, 0:1]
                )

                # --- S4: T4 = Z^T CASP ---
                t4 = mid.tile(shp, mm_dtype, tag="t4")
                for m in range(NCHUNK):
                    b = ps.tile([128, N], f32, tag="bank")
                    acc_mm(b, [(zt, caspt)], m)
                    nc.scalar.copy(t4[:, m, :], b[:, :])

                # --- S5: out = T4^T CASP ---
                ot = outp.tile(shp, f32, tag="ot")
                for m in range(NCHUNK):
                    b = ps.tile([128, N], f32, tag="bank")
                    acc_mm(b, [(t4, caspt)], m)
                    nc.scalar.copy(ot[:, m, :], b[:, :])
                nc.sync.dma_start(out=_plane(out[j]), in_=ot[:])

    legalize_waits(nc)
    return nc


# ---------------------------------------------------------------------------
# host wrapper
# ---------------------------------------------------------------------------
_nc_cache = {}


def _get_nc(n_img, mm_dtype):
    key = (n_img, str(mm_dtype))
    if key not in _nc_cache:
        _nc_cache[key] = build_nc(n_img, mm_dtype)
    return _nc_cache[key]


def _host_consts():
    n = np.arange(N)
    ang = 2.0 * np.pi * np.outer(n, n) / N
    casp = np.cos(ang) + np.sin(ang)
    rev = np.zeros((N, N))
    rev[n, (-n) % N] = 1.0
    return np.stack([casp, rev]).astype(np.float32)


def _filter_planes(H_real, H_imag):
    """E1, E2, Rv(E3), Rv(E4) for the DHT convolution theorem (1/N^2 in)."""
    Hc = np.asarray(H_real, np.float64) + 1j * np.asarray(H_imag, np.float64)
    idx = (-np.arange(N)) % N
    G = 0.5 * (Hc + np.conj(Hc[np.ix_(idx, idx)]))
    ReH, ImH = np.real(G), np.imag(G)

    def Ru(a):
        return a[idx, :]

    def Rv(a):
        return a[:, idx]

    sc = 1.0 / (N * N)
    E1 = 0.5 * (Ru(ReH) + ReH) * sc
    E2 = 0.5 * (Ru(ImH) - ImH) * sc
    E3 = -0.5 * (Ru(ImH) + ImH) * sc
    E4 = 0.5 * (Ru(ReH) - ReH) * sc
    return np.stack([E1, E2, Rv(E3), Rv(E4)]).astype(np.float32)


def kernel(x, H_real, H_imag):
    x = np.asarray(x, dtype=np.float32)
    B, C, H, W = x.shape
    assert (H, W) == (N, N) and B * C == N_CORES * IMG_PER_CORE

    emat = _filter_planes(H_real, H_imag)
    wmat = _host_consts()

    xf = np.ascontiguousarray(x.reshape(B * C, N, N))
    nc = _get_nc(IMG_PER_CORE, MM_DTYPE)
    in_maps = [
        {
            "x": xf[i * IMG_PER_CORE : (i + 1) * IMG_PER_CORE],
            "wmat": wmat,
            "emat": emat,
        }
        for i in range(N_CORES)
    ]
    res = run_bass_kernel_spmd(nc, in_maps, list(range(N_CORES)))
    outs = [res.results[i]["out"] for i in range(N_CORES)]
    return np.concatenate(outs, axis=0).reshape(B, C, N, N)


# revision 8
# speedup vs baseline: 1.1877x; 1.1877x over previous
"""Trainium2 kernel for PhysicalDiffraction:
    out = real(ifft2(fft2(x) * (H_real + i*H_imag)))   x: [8,16,512,512] f32

Method
------
The op is a real->real linear map, so the filter can be Hermitian-
symmetrized: G = (Hc + conj(Hc[-u,-v]))/2 gives identical output and
makes ifft2(fft2(x)*G) exactly real.  That lets us pack TWO real images
into ONE complex image z = x_a + i*x_b; after filtering, real/imag parts
are the two outputs.  FFTs are done as DFT matrix multiplications
(F = C - iS, symmetric), using the lhsT=data trick so each stage's
matmul both transforms and transposes, chaining with no explicit
transposes:

  S1: Y1 = z^T F          S2: Y2 = Y1^T F      (= fft2(z))
  S3: Z  = Y2 * G/N^2     (pointwise complex, on VectorE)
  S4: Y3 = Z^T conj(F)    S5: Y4 = Y3^T conj(F)  (= ifft2(...)*N^2)

Sharding: batch*channel = 128 images, 16 per NeuronCore (data-parallel,
no communication), 8 complex images per core.

The walrus build here accepts only ONE semaphore wait per instruction;
Tile emits more.  `legalize_waits` splits excess waits onto same-engine
NoOps inserted just before the instruction (the engine sequencer stalls
there instead), which preserves semantics.
"""

import sys

for _p in ("/opt/trn_rl_repo", "/root/.axon_site/_ro/trn_rl_repo"):
    if _p not in sys.path:
        sys.path.append(_p)

import numpy as np
from concourse import bass, mybir
from concourse.tile import TileContext
from concourse.bass_utils import run_bass_kernel_spmd

N = 512
NCHUNK = N // 128  # 4
N_CORES = 8
IMG_PER_CORE = 16

# matmul dtype: mybir.dt.float32r (fast fp32-ish) or mybir.dt.bfloat16
MM_DTYPE = mybir.dt.float32r


# ---------------------------------------------------------------------------
# wait legalizer
# ---------------------------------------------------------------------------
_counter = [0]


def _fresh_name():
    _counter[0] += 1
    return f"I-waitfix-{_counter[0]}"


def legalize_waits(nc, limit=1):
    for fn in nc.m.functions:
        for blk in fn.blocks:
            out = []
            changed = False
            for inst in blk.instructions:
                si = inst.sync_info
                waits = list(si.on_wait) if si is not None and si.on_wait else []
                if len(waits) > limit:
                    excess = waits[: len(waits) - limit]
                    keep = waits[len(waits) - limit :]
                    for w in excess:
                        out.append(
                            mybir.InstNoOp(
                                name=_fresh_name(),
                                engine=inst.engine,
                                ins=[],
                                outs=[],
                                sync_info=mybir.SyncInfo(on_wait=[w], on_update=[]),
                            )
                        )
                    inst.sync_info = mybir.SyncInfo(
                        on_wait=keep,
                        on_update=list(si.on_update) if si.on_update else [],
                    )
                    changed = True
                out.append(inst)
            if changed:
                blk.instructions = out
    return nc


# ---------------------------------------------------------------------------
# bass program: one core, n_img images (even)
# ---------------------------------------------------------------------------
def _plane(dram_ap):
    """[512,512] DRAM view -> [128, 4, 512] partition-major AP."""
    return dram_ap.rearrange("(k p) w -> p k w", p=128)


def build_nc(n_img=IMG_PER_CORE, mm_dtype=MM_DTYPE):
    f32 = mybir.dt.float32
    nc = bass.Bass()
    x = nc.declare_dram_parameter("x", [n_img, N, N], f32, isOutput=False)
    cmat = nc.declare_dram_parameter("cmat", [N, N], f32, isOutput=False)
    smat = nc.declare_dram_parameter("smat", [N, N], f32, isOutput=False)
    nsmat = nc.declare_dram_parameter("nsmat", [N, N], f32, isOutput=False)
    gre = nc.declare_dram_parameter("gre", [N, N], f32, isOutput=False)
    gim = nc.declare_dram_parameter("gim", [N, N], f32, isOutput=False)
    out = nc.declare_dram_parameter("out", [n_img, N, N], f32, isOutput=True)

    shp = [128, NCHUNK, N]

    with TileContext(nc) as tc:
        with (
            tc.tile_pool(name="wts", bufs=1) as wts,
            tc.tile_pool(name="stg", bufs=2) as stg,
            tc.tile_pool(name="zin", bufs=2) as zin,
            tc.tile_pool(name="mid", bufs=1) as mid,
            tc.tile_pool(name="tmp", bufs=2) as tmp,
            tc.tile_pool(name="outp", bufs=2) as outp,
            tc.tile_pool(name="ps", bufs=4, space="PSUM") as ps,
        ):
            # --- one-time: DFT weights (rounded to mm dtype) + filter ---
            ct = wts.tile(shp, mm_dtype, tag="ct")
            st = wts.tile(shp, mm_dtype, tag="st")
            nst = wts.tile(shp, mm_dtype, tag="nst")
            gr = wts.tile(shp, f32, tag="gr")
            gi = wts.tile(shp, f32, tag="gi")
            for src, dst in ((cmat, ct), (smat, st), (nsmat, nst)):
                s = stg.tile(shp, f32, tag="stg")
                nc.sync.dma_start(out=s[:], in_=_plane(src))
                nc.vector.tensor_copy(dst[:], s[:])
            nc.sync.dma_start(out=gr[:], in_=_plane(gre))
            nc.sync.dma_start(out=gi[:], in_=_plane(gim))

            rhs_of = {"c": ct, "s": st, "n": nst}

            def mm_stage(lhs_r, lhs_i, terms_r, terms_i, evac):
                """One transform stage: per m-tile accumulate the r/i PSUM
                banks over terms x k-chunks, then hand to evac."""
                for m in range(NCHUNK):
                    pr = ps.tile([128, N], f32, tag="pr")
                    pi = ps.tile([128, N], f32, tag="pi")
                    for bank, terms in ((pr, terms_r), (pi, terms_i)):
                        nterm = len(terms)
                        for ti, (lhs_sel, rhs_sel) in enumerate(terms):
                            lhs = lhs_r if lhs_sel == "r" else lhs_i
                            rhs = rhs_of[rhs_sel]
                            for k in range(NCHUNK):
                                nc.tensor.matmul(
                                    bank[:, :],
                                    lhs[:, k, m * 128 : (m + 1) * 128],
                                    rhs[:, k, :],
                                    start=(ti == 0 and k == 0),
                                    stop=(ti == nterm - 1 and k == NCHUNK - 1),
                                )
                    evac(m, pr, pi)

            FWD = dict(terms_r=[("r", "c"), ("i", "s")],
                       terms_i=[("i", "c"), ("r", "n")])
            INV = dict(terms_r=[("r", "c"), ("i", "n")],
                       terms_i=[("r", "s"), ("i", "c")])

            for j in range(n_img // 2):
                # --- stage inputs (cast/round to mm dtype on DVE) ---
                zr = zin.tile(shp, mm_dtype, tag="zr")
                zi = zin.tile(shp, mm_dtype, tag="zi")
                for dst, img in ((zr, 2 * j), (zi, 2 * j + 1)):
                    s = stg.tile(shp, f32, tag="stg")
                    nc.sync.dma_start(out=s[:], in_=_plane(x[img]))
                    nc.vector.tensor_copy(dst[:], s[:])

                # --- S1: Y1 = z^T F ---
                y1r = mid.tile(shp, mm_dtype, tag="y1r")
                y1i = mid.tile(shp, mm_dtype, tag="y1i")

                def evac_s1(m, pr, pi):
                    nc.scalar.copy(y1r[:, m, :], pr[:, :])
                    nc.scalar.copy(y1i[:, m, :], pi[:, :])

                mm_stage(zr, zi, evac=evac_s1, **FWD)

                # --- S2 + S3: Y2 = Y1^T F, then Z = Y2 * G (fused evac) ---
                zrt = mid.tile(shp, mm_dtype, tag="zrt")
                zit = mid.tile(shp, mm_dtype, tag="zit")

                def evac_gmul(m, pr, pi):
                    t1 = tmp.tile([128, N], f32, tag="t1")
                    t2 = tmp.tile([128, N], f32, tag="t2")
                    nc.vector.tensor_mul(t1[:, :], pr[:, :], gr[:, m, :])
                    nc.vector.tensor_mul(t2[:, :], pi[:, :], gi[:, m, :])
                    nc.vector.tensor_sub(zrt[:, m, :], t1[:, :], t2[:, :])
                    t3 = tmp.tile([128, N], f32, tag="t1")
                    t4 = tmp.tile([128, N], f32, tag="t2")
                    nc.vector.tensor_mul(t3[:, :], pr[:, :], gi[:, m, :])
                    nc.vector.tensor_mul(t4[:, :], pi[:, :], gr[:, m, :])
                    nc.vector.tensor_add(zit[:, m, :], t3[:, :], t4[:, :])

                mm_stage(y1r, y1i, evac=evac_gmul, **FWD)

                # --- S4: Y3 = Z^T conj(F) ---
                y3r = mid.tile(shp, mm_dtype, tag="y3r")
                y3i = mid.tile(shp, mm_dtype, tag="y3i")

                def evac_s4(m, pr, pi):
                    nc.scalar.copy(y3r[:, m, :], pr[:, :])
                    nc.scalar.copy(y3i[:, m, :], pi[:, :])

                mm_stage(zrt, zit, evac=evac_s4, **INV)

                # --- S5: Y4 = Y3^T conj(F); real -> img 2j, imag -> 2j+1 ---
                or_t = outp.tile(shp, f32, tag="or")
                oi_t = outp.tile(shp, f32, tag="oi")

                def evac_out(m, pr, pi):
                    nc.scalar.copy(or_t[:, m, :], pr[:, :])
                    nc.scalar.copy(oi_t[:, m, :], pi[:, :])

                mm_stage(y3r, y3i, evac=evac_out, **INV)
                nc.sync.dma_start(out=_plane(out[2 * j]), in_=or_t[:])
                nc.sync.dma_start(out=_plane(out[2 * j + 1]), in_=oi_t[:])

    legalize_waits(nc)
    return nc


# ---------------------------------------------------------------------------
# host wrapper
# ---------------------------------------------------------------------------
_nc_cache = {}


def _get_nc(n_img, mm_dtype):
    key = (n_img, str(mm_dtype))
    if key not in _nc_cache:
        _nc_cache[key] = build_nc(n_img, mm_dtype)
    return _nc_cache[key]


def _host_consts():
    u = np.arange(N)
    ang = 2.0 * np.pi * np.outer(u, u) / N
    cmat = np.cos(ang).astype(np.float32)
    smat = np.sin(ang).astype(np.float32)
    return cmat, smat, (-smat).astype(np.float32)


def _filter_planes(H_real, H_imag):
    Hc = np.asarray(H_real, np.float64) + 1j * np.asarray(H_imag, np.float64)
    idx = (-np.arange(N)) % N
    G = 0.5 * (Hc + np.conj(Hc[np.ix_(idx, idx)])) / (N * N)
    return np.real(G).astype(np.float32), np.imag(G).astype(np.float32)


def kernel(x, H_real, H_imag):
    x = np.asarray(x, dtype=np.float32)
    B, C, H, W = x.shape
    assert (H, W) == (N, N) and B * C == N_CORES * IMG_PER_CORE

    gre, gim = _filter_planes(H_real, H_imag)
    cmat, smat, nsmat = _host_consts()

    xf = np.ascontiguousarray(x.reshape(B * C, N, N))
    nc = _get_nc(IMG_PER_CORE, MM_DTYPE)
    in_maps = [
        {
            "x": xf[i * IMG_PER_CORE : (i + 1) * IMG_PER_CORE],
            "cmat": cmat,
            "smat": smat,
            "nsmat": nsmat,
            "gre": gre,
            "gim": gim,
        }
        for i in range(N_CORES)
    ]
    res = run_bass_kernel_spmd(nc, in_maps, list(range(N_CORES)))
    outs = [res.results[i]["out"] for i in range(N_CORES)]
    return np.concatenate(outs, axis=0).reshape(B, C, N, N)


# revision 11
# speedup vs baseline: 1.7361x; 1.4617x over previous
"""Trainium2 kernel for PhysicalDiffraction:
    out = real(ifft2(fft2(x) * (H_real + i*H_imag)))   x: [8,16,512,512] f32

Method
------
The op is a real->real linear map, so the filter can be Hermitian-
symmetrized: G = (Hc + conj(Hc[-u,-v]))/2 gives identical output and
makes ifft2(fft2(x)*G) exactly real.  That lets us pack TWO real images
into ONE complex image z = x_a + i*x_b; after filtering, real/imag parts
are the two outputs.  FFTs are done as DFT matrix multiplications
(F = C - iS, symmetric), using the lhsT=data trick so each stage's
matmul both transforms and transposes, chaining with no explicit
transposes:

  S1: Y1 = z^T F          S2: Y2 = Y1^T F      (= fft2(z))
  S3: Z  = Y2 * G/N^2     (pointwise complex, on VectorE)
  S4: Y3 = Z^T conj(F)    S5: Y4 = Y3^T conj(F)  (= ifft2(...)*N^2)

Sharding: batch*channel = 128 images, 16 per NeuronCore (data-parallel,
no communication), 8 complex images per core.

The walrus build here accepts only ONE semaphore wait per instruction;
Tile emits more.  `legalize_waits` splits excess waits onto same-engine
NoOps inserted just before the instruction (the engine sequencer stalls
there instead), which preserves semantics.
"""

import sys

for _p in ("/opt/trn_rl_repo", "/root/.axon_site/_ro/trn_rl_repo"):
    if _p not in sys.path:
        sys.path.append(_p)

import numpy as np
from concourse import bass, mybir
from concourse.tile import TileContext
from concourse.bass_utils import run_bass_kernel_spmd

N = 512
NCHUNK = N // 128  # 4
N_CORES = 8
IMG_PER_CORE = 16

# matmul dtype: mybir.dt.float32r (fast fp32-ish) or mybir.dt.bfloat16
MM_DTYPE = mybir.dt.float32r


# ---------------------------------------------------------------------------
# wait legalizer
# ---------------------------------------------------------------------------
_counter = [0]


def _fresh_name():
    _counter[0] += 1
    return f"I-waitfix-{_counter[0]}"


def legalize_waits(nc, limit=1):
    for fn in nc.m.functions:
        for blk in fn.blocks:
            out = []
            changed = False
            for inst in blk.instructions:
                si = inst.sync_info
                waits = list(si.on_wait) if si is not None and si.on_wait else []
                if len(waits) > limit:
                    excess = waits[: len(waits) - limit]
                    keep = waits[len(waits) - limit :]
                    for w in excess:
                        out.append(
                            mybir.InstNoOp(
                                name=_fresh_name(),
                                engine=inst.engine,
                                ins=[],
                                outs=[],
                                sync_info=mybir.SyncInfo(on_wait=[w], on_update=[]),
                            )
                        )
                    inst.sync_info = mybir.SyncInfo(
                        on_wait=keep,
                        on_update=list(si.on_update) if si.on_update else [],
                    )
                    changed = True
                out.append(inst)
            if changed:
                blk.instructions = out
    return nc


# ---------------------------------------------------------------------------
# bass program: one core, n_img images (even)
# ---------------------------------------------------------------------------
def _plane(dram_ap):
    """[512,512] DRAM view -> [128, 4, 512] partition-major AP."""
    return dram_ap.rearrange("(k p) w -> p k w", p=128)


def build_nc(n_img=IMG_PER_CORE, mm_dtype=MM_DTYPE):
    f32 = mybir.dt.float32
    nc = bass.Bass()
    x = nc.declare_dram_parameter("x", [n_img, N, N], f32, isOutput=False)
    cmat = nc.declare_dram_parameter("cmat", [N, N], f32, isOutput=False)
    smat = nc.declare_dram_parameter("smat", [N, N], f32, isOutput=False)
    nsmat = nc.declare_dram_parameter("nsmat", [N, N], f32, isOutput=False)
    gre = nc.declare_dram_parameter("gre", [N, N], f32, isOutput=False)
    gim = nc.declare_dram_parameter("gim", [N, N], f32, isOutput=False)
    out = nc.declare_dram_parameter("out", [n_img, N, N], f32, isOutput=True)

    shp = [128, NCHUNK, N]

    with TileContext(nc) as tc:
        with (
            tc.tile_pool(name="wts", bufs=1) as wts,
            tc.tile_pool(name="stg", bufs=2) as stg,
            tc.tile_pool(name="zin", bufs=2) as zin,
            tc.tile_pool(name="mid", bufs=1) as mid,
            tc.tile_pool(name="tmp", bufs=2) as tmp,
            tc.tile_pool(name="outp", bufs=2) as outp,
            tc.tile_pool(name="ps", bufs=4, space="PSUM") as ps,
        ):
            # --- one-time: DFT weights (rounded to mm dtype) + filter ---
            ct = wts.tile(shp, mm_dtype, tag="ct")
            st = wts.tile(shp, mm_dtype, tag="st")
            nst = wts.tile(shp, mm_dtype, tag="nst")
            gr = wts.tile(shp, f32, tag="gr")
            gi = wts.tile(shp, f32, tag="gi")
            for src, dst in ((cmat, ct), (smat, st), (nsmat, nst)):
                s = stg.tile(shp, f32, tag="stg")
                nc.sync.dma_start(out=s[:], in_=_plane(src))
                nc.vector.tensor_copy(dst[:], s[:])
            nc.sync.dma_start(out=gr[:], in_=_plane(gre))
            nc.sync.dma_start(out=gi[:], in_=_plane(gim))

            rhs_of = {"c": ct, "s": st, "n": nst}

            def mm_stage(lhs_r, lhs_i, terms_r, terms_i, evac):
                """One transform stage: per m-tile accumulate the r/i PSUM
                banks over terms x k-chunks, then hand to evac."""
                for m in range(NCHUNK):
                    pr = ps.tile([128, N], f32, tag="pr")
                    pi = ps.tile([128, N], f32, tag="pi")
                    for bank, terms in ((pr, terms_r), (pi, terms_i)):
                        nterm = len(terms)
                        for ti, (lhs_sel, rhs_sel) in enumerate(terms):
                            lhs = lhs_r if lhs_sel == "r" else lhs_i
                            rhs = rhs_of[rhs_sel]
                            for k in range(NCHUNK):
                                nc.tensor.matmul(
                                    bank[:, :],
                                    lhs[:, k, m * 128 : (m + 1) * 128],
                                    rhs[:, k, :],
                                    start=(ti == 0 and k == 0),
                                    stop=(ti == nterm - 1 and k == NCHUNK - 1),
                                )
                    evac(m, pr, pi)

            FWD = dict(terms_r=[("r", "c"), ("i", "s")],
                       terms_i=[("i", "c"), ("r", "n")])
            INV = dict(terms_r=[("r", "c"), ("i", "n")],
                       terms_i=[("r", "s"), ("i", "c")])

            npair = n_img // 2
            st = {}  # per-pair tile state

            def stage_in(p):
                zr = zin.tile(shp, mm_dtype, tag="zr", name=f"zr{p}")
                zi = zin.tile(shp, mm_dtype, tag="zi", name=f"zi{p}")
                for dst, img in ((zr, 2 * p), (zi, 2 * p + 1)):
                    s = stg.tile(shp, f32, tag="stg", name=f"xs{img}")
                    nc.sync.dma_start(out=s[:], in_=_plane(x[img]))
                    nc.vector.tensor_copy(dst[:], s[:])
                st[p] = {"zr": zr, "zi": zi}

            def s1(p):
                y1r = mid.tile(shp, mm_dtype, tag="y1r", name=f"y1r{p}")
                y1i = mid.tile(shp, mm_dtype, tag="y1i", name=f"y1i{p}")

                def evac(m, pr, pi):
                    nc.scalar.copy(y1r[:, m, :], pr[:, :])
                    nc.scalar.copy(y1i[:, m, :], pi[:, :])

                mm_stage(st[p]["zr"], st[p]["zi"], evac=evac, **FWD)
                st[p]["y1r"], st[p]["y1i"] = y1r, y1i

            def s2(p):
                zrt = mid.tile(shp, mm_dtype, tag="zrt", name=f"zrt{p}")
                zit = mid.tile(shp, mm_dtype, tag="zit", name=f"zit{p}")

                def evac(m, pr, pi):
                    t1 = tmp.tile([128, N], f32, tag="t1", name=f"t1_{p}_{m}")
                    t2 = tmp.tile([128, N], f32, tag="t2", name=f"t2_{p}_{m}")
                    nc.vector.tensor_mul(t1[:, :], pr[:, :], gr[:, m, :])
                    nc.vector.tensor_mul(t2[:, :], pi[:, :], gi[:, m, :])
                    nc.vector.tensor_sub(zrt[:, m, :], t1[:, :], t2[:, :])
                    t3 = tmp.tile([128, N], f32, tag="t1", name=f"t3_{p}_{m}")
                    t4 = tmp.tile([128, N], f32, tag="t2", name=f"t4_{p}_{m}")
                    nc.vector.tensor_mul(t3[:, :], pr[:, :], gi[:, m, :])
                    nc.vector.tensor_mul(t4[:, :], pi[:, :], gr[:, m, :])
                    nc.vector.tensor_add(zit[:, m, :], t3[:, :], t4[:, :])

                mm_stage(st[p]["y1r"], st[p]["y1i"], evac=evac, **FWD)
                st[p]["zrt"], st[p]["zit"] = zrt, zit

            def s4(p):
                y3r = mid.tile(shp, mm_dtype, tag="y3r", name=f"y3r{p}")
                y3i = mid.tile(shp, mm_dtype, tag="y3i", name=f"y3i{p}")

                def evac(m, pr, pi):
                    nc.scalar.copy(y3r[:, m, :], pr[:, :])
                    nc.scalar.copy(y3i[:, m, :], pi[:, :])

                mm_stage(st[p]["zrt"], st[p]["zit"], evac=evac, **INV)
                st[p]["y3r"], st[p]["y3i"] = y3r, y3i

            def s5(p):
                or_t = outp.tile(shp, f32, tag="or", name=f"or{p}")
                oi_t = outp.tile(shp, f32, tag="oi", name=f"oi{p}")

                def evac(m, pr, pi):
                    nc.scalar.copy(or_t[:, m, :], pr[:, :])
                    nc.scalar.copy(oi_t[:, m, :], pi[:, :])

                mm_stage(st[p]["y3r"], st[p]["y3i"], evac=evac, **INV)
                nc.sync.dma_start(out=_plane(out[2 * p]), in_=or_t[:])
                nc.sync.dma_start(out=_plane(out[2 * p + 1]), in_=oi_t[:])
                del st[p]

            # software pipeline: another pair's matmuls fill each
            # stage-boundary evacuation bubble
            stage_in(0)
            for p in range(npair):
                s1(p)
                if p >= 1:
                    s4(p - 1)
                s2(p)
                if p >= 1:
                    s5(p - 1)
                if p + 1 < npair:
                    stage_in(p + 1)
            s4(npair - 1)
            s5(npair - 1)

    legalize_waits(nc)
    return nc


# ---------------------------------------------------------------------------
# host wrapper
# ---------------------------------------------------------------------------
_nc_cache = {}


def _get_nc(n_img, mm_dtype):
    key = (n_img, str(mm_dtype))
    if key not in _nc_cache:
        _nc_cache[key] = build_nc(n_img, mm_dtype)
    return _nc_cache[key]


def _host_consts():
    u = np.arange(N)
    ang = 2.0 * np.pi * np.outer(u, u) / N
    cmat = np.cos(ang).astype(np.float32)
    smat = np.sin(ang).astype(np.float32)
    return cmat, smat, (-smat).astype(np.float32)


def _filter_planes(H_real, H_imag):
    Hc = np.asarray(H_real, np.float64) + 1j * np.asarray(H_imag, np.float64)
    idx = (-np.arange(N)) % N
    G = 0.5 * (Hc + np.conj(Hc[np.ix_(idx, idx)])) / (N * N)
    return np.real(G).astype(np.float32), np.imag(G).astype(np.float32)


def kernel(x, H_real, H_imag):
    x = np.asarray(x, dtype=np.float32)
    B, C, H, W = x.shape
    assert (H, W) == (N, N) and B * C == N_CORES * IMG_PER_CORE

    gre, gim = _filter_planes(H_real, H_imag)
    cmat, smat, nsmat = _host_consts()

    xf = np.ascontiguousarray(x.reshape(B * C, N, N))
    nc = _get_nc(IMG_PER_CORE, MM_DTYPE)
    in_maps = [
        {
            "x": xf[i * IMG_PER_CORE : (i + 1) * IMG_PER_CORE],
            "cmat": cmat,
            "smat": smat,
            "nsmat": nsmat,
            "gre": gre,
            "gim": gim,
        }
        for i in range(N_CORES)
    ]
    res = run_bass_kernel_spmd(nc, in_maps, list(range(N_CORES)))
    outs = [res.results[i]["out"] for i in range(N_CORES)]
    return np.concatenate(outs, axis=0).reshape(B, C, N, N)
